# revision 30
# baseline (speedup 1.0000x reference)
"""Trainium2 Bass kernel for nn_CAModel (neural cellular automaton step).

Per-core (8-way batch-parallel, 2 images/core) bf16 pipeline, v2:
  - x loaded twice: packed layout xf (partition = u*16+c, halo rows) for the
    sobel build + epilogue, and directly into the pair-layout percep tile P.
  - percep pair layout P: partitions 0:16 x / 16:32 dwx / 32:48 dwy for the
    even unit of a pair, same +64 for the odd unit; free = (img, pair, px).
    L1 is then ONE K=48 matmul per (unit, 512px tile) with a single shared
    stationary weight (w1 duplicated into rows 0:48 and 64:112) and 2-way
    row-group concurrency -- vs 3 zero-padded K=32 matmuls before.
  - sobel build split: vertical passes (ps, v1) on GpSimd, rest on DVE,
    chunked 6 rows at a time, repacked into P via SBUF->SBUF DMAs.
  - L2 dense K=128; L3 col-tiled M=32 with zero-padded w3 halves.
  - evacuations split ACT/DVE; L3 evac fused with (+b3)*update_mask on DVE,
    then x+dx on GpSimd per tile.
  - life masks via stripe-packed 3x3 maxpool on GpSimd; bf16 output store.
"""

import numpy as np
import ml_dtypes
import concourse.bass as bass
import concourse.tile as tile
from concourse import bacc, mybir

AF = mybir.ActivationFunctionType
OP = mybir.AluOpType
f16 = mybir.dt.bfloat16
f32 = mybir.dt.float32

BL, C, H, W = 2, 16, 192, 192   # per-core images
U, RPU = 8, 24                  # row-block units per image, rows per unit
FPI = RPU * W                   # 4608 free elems per (img,unit)
NT, TS = 9, 512                 # tiles per (img,unit), pixels per tile
HID = 128
FHI = 26 * W                    # 4992 halo'd elems per img in xf

WARMN = 90   # PE warmup matmuls covering the front runway
RC = 6       # sobel build chunk rows

# engine for L1/L2 evacuations, keyed by unit: 'a' = ACT, 'v' = DVE
L1_ENG = "aavaaava"
L2_ENG = "aavaavaa"


def build_nc():
    nc = bacc.Bacc("TRN2", target_bir_lowering=False, debug=False)

    x_d = nc.dram_tensor("x", [BL, C, H, W], f16, kind="ExternalInput")
    fn_d = nc.dram_tensor("fn", [BL, H, W], f16, kind="ExternalInput")  # host-side umask {0,1}
    wm_d = nc.dram_tensor("wm", [128, 320], f16, kind="ExternalInput")
    bm_d = nc.dram_tensor("bm", [128, 3], f32, kind="ExternalInput")
    out_d = nc.dram_tensor("out", [BL, C, H, W], f16, kind="ExternalOutput")

    with tile.TileContext(nc) as tc:
        with (
            tc.tile_pool(name="const", bufs=1) as const,
            tc.tile_pool(name="xf", bufs=1) as xfp,
            tc.tile_pool(name="pp", bufs=1) as ppool,
            tc.tile_pool(name="dwc1", bufs=1) as dwc1,
            tc.tile_pool(name="dwc2", bufs=2) as dwc2,
            tc.tile_pool(name="msk", bufs=1) as mskp,
            tc.tile_pool(name="strp", bufs=1) as strp,
            tc.tile_pool(name="h1p", bufs=10) as h1p,
            tc.tile_pool(name="h2p", bufs=9) as h2p,
            tc.tile_pool(name="dram", bufs=1, space="DRAM") as dramp,
            tc.tile_pool(name="dwd", bufs=2, space="DRAM") as dwdp,
            tc.tile_pool(name="pz12", bufs=3, space="PSUM") as pz12,
            tc.tile_pool(name="pz3", bufs=2, space="PSUM") as pz3,
        ):
            # ---- load x bf16 first (3 DMAs per image: body + 2 halo rows) ----
            xf = xfp.tile([128, BL, RPU + 2, W], f16)
            nc.vector.memset(xf[0:32, :, 0:1, :], 0.0)
            nc.vector.memset(xf[96:128, :, 25:26, :], 0.0)

            def load_xf(img):
                for u in range(U):
                    lo = max(0, u * RPU - 1)
                    hi = min(H, u * RPU + RPU + 1)
                    rb0 = 1 - (u * RPU - lo)
                    eng = nc.scalar if u % 2 == 0 else nc.sync
                    eng.dma_start(
                        xf[u * 16:(u + 1) * 16, img, rb0:rb0 + (hi - lo), :],
                        x_d.ap()[img, :, lo:hi, :],
                    )

            load_xf(0)

            # ---- constants (merged loads) ----
            wt = const.tile([128, 320], f16)
            nc.scalar.dma_start(wt[:], wm_d.ap())
            bt = const.tile([128, 3], f32)
            nc.scalar.dma_start(bt[:], bm_d.ap())
            w1dup = wt[:, 0:128]
            w2t = wt[:, 128:256]
            w3t = wt[:, 256:320]
            b1c = bt[:, 0:1]
            b2c = bt[:, 1:2]
            b3c = bt[:, 2:3]
            dummy = const.tile([128, TS], f16)
            nc.vector.memset(dummy[:], 0.0)
            wdum = const.tile([128, 128], f16)
            nc.vector.memset(wdum[:], 0.0)

            xf_flat0 = xf[:].rearrange("p i r w -> p (i r w)")

            # ---- percep pair-layout tile + direct x loads ----
            P = ppool.tile([128, BL, 4, FPI], f16)

            def load_px(img):
                xsrc = x_d.ap()[img].rearrange("c (j q r) w -> q c j r w", q=2, r=RPU)
                for q in range(2):
                    eng = nc.scalar if q == 0 else nc.sync
                    eng.dma_start(P[64 * q:64 * q + 16, img, :, :], xsrc[q])

            load_px(0)
            load_xf(1)
            load_px(1)

            # ---- update mask (host-computed {0,1}), broadcast over channels ----
            umasks = [mskp.tile([128, FPI], f16, tag=f"um{i}", name=f"um{i}") for i in range(BL)]

            def emit_umask(img):
                for u in range(U):
                    s = fn_d.ap()[img, u * RPU:(u + 1) * RPU, :]
                    s = s.rearrange("a b -> (a b)").partition_broadcast(16)
                    nc.sync.dma_start(umasks[img][u * 16:(u + 1) * 16], s)

            # ---- PE warmup runway ----
            zw = pz3.tile([128, TS], f32, tag="z3", name="zw")
            for _ in range(WARMN):
                nc.tensor.matmul(zw[:, :], wdum[:, :], dummy[:, :], start=True, stop=True)

            # ---- sobel build chunk (rc rows, all units/channels) + repack into P ----
            # flat-row trick: main passes run over flattened (r w); the row-seam
            # entries they corrupt are exactly the border columns, which the
            # border ops recompute afterwards.
            def emit_chunk(img, r0, rc):
                RC = rc
                F = RC * W
                xb = img * FHI + W  # xf_flat offset of image row 0
                ps = dwc1.tile([128, (RC + 1) * W], f16, tag="ps")
                nc.gpsimd.tensor_add(
                    ps[:], xf_flat0[:, xb + (r0 - 1) * W:xb + (r0 + RC) * W],
                    xf_flat0[:, xb + r0 * W:xb + (r0 + RC + 1) * W]
                )
                v1f = dwc1.tile([128, F], f16, tag="v1")
                nc.gpsimd.tensor_add(v1f[:], ps[:, 0:F], ps[:, W:F + W])
                v1 = v1f[:].rearrange("p (r w) -> p r w", w=W)
                v2f = dwc1.tile([128, F], f16, tag="v2")
                nc.vector.tensor_sub(
                    v2f[:], xf_flat0[:, xb + (r0 + 1) * W:xb + (r0 + RC + 1) * W],
                    xf_flat0[:, xb + (r0 - 1) * W:xb + (r0 + RC - 1) * W]
                )
                v2 = v2f[:].rearrange("p (r w) -> p r w", w=W)
                qsf = dwc1.tile([128, F], f16, tag="qs")
                nc.vector.tensor_add(qsf[:, 0:F - 1], v2f[:, 0:F - 1], v2f[:, 1:F])
                qs = qsf[:].rearrange("p (r w) -> p r w", w=W)
                dw = dwc2.tile([128, 2, RC, W], f16, tag="dw")  # [:,0]=dwx [:,1]=dwy
                dwf = dw[:].rearrange("p m r w -> p m (r w)")
                # dwx = v1[c+1] - v1[c-1]; flat main, then borders (zero-pad semantics)
                nc.vector.tensor_sub(dwf[:, 0, 1:F - 1], v1f[:, 2:F], v1f[:, 0:F - 2])
                nc.vector.tensor_copy(dw[:, 0, :, 0:1], v1[:, :, 1:2])
                nc.vector.tensor_scalar_mul(dw[:, 0, :, 191:192], v1[:, :, 190:191], -1.0)
                # dwy = qs[c-1] + qs[c]; flat main, then borders
                nc.vector.tensor_add(dwf[:, 1, 1:F - 1], qsf[:, 0:F - 2], qsf[:, 1:F - 1])
                nc.gpsimd.tensor_add(dw[:, 1, :, 0:1], qs[:, :, 0:1], v2[:, :, 0:1])
                nc.vector.tensor_add(dw[:, 1, :, 191:192], qs[:, :, 190:191], v2[:, :, 191:192])
                # repack via DRAM bounce: packed (u*16+c) partitions -> pair layout
                dwd = dwdp.tile([128, 2, RC * W], f16, tag="dwd", name=f"dwd_{img}_{r0}")
                nc.sync.dma_start(dwd[:], dw[:].rearrange("p m r w -> p m (r w)"))
                hsrc = dwd[:].rearrange("(j q c) m x -> q m c j x", q=2, c=16)
                for parity in range(2):
                    for m, pbase in ((0, 16), (1, 32)):
                        nc.sync.dma_start(
                            P[pbase + 64 * parity:pbase + 64 * parity + 16,
                              img, :, r0 * W:(r0 + RC) * W],
                            hsrc[parity, m],
                        )

            # ---- pre-life maxpool (stripe layout: partition = img*64 + s, 3 rows each) ----
            xf_r = xf[:].rearrange("(u c) i r w -> u c i r w", c=16)

            def emit_m2pre():
                al = strp.tile([128, 3, W], f16, tag="al_s")
                for img in range(BL):
                    s = xf_r[:, 3, img, 1:25, :].rearrange("u (s r) w -> u s (r w)", r=3)
                    nc.sync.dma_start(al[img * 64:img * 64 + 32, :, :], s[0:4])
                    nc.scalar.dma_start(al[img * 64 + 32:(img + 1) * 64, :, :], s[4:8])
                pm = strp.tile([128, 3, 191], f16, tag="pm_s")
                nc.vector.tensor_max(pm[:], al[:, :, 0:191], al[:, :, 1:192])
                m1 = strp.tile([128, 3, W], f16, tag="m1_s")
                nc.vector.tensor_max(m1[:, :, 1:191], pm[:, :, 0:190], pm[:, :, 1:191])
                nc.vector.tensor_copy(m1[:, :, 0:1], pm[:, :, 0:1])
                nc.vector.tensor_copy(m1[:, :, 191:192], pm[:, :, 190:191])
                hh = strp.tile([128, 2, W], f16, tag="hh_s")
                nc.sync.dma_start(hh[1:128, 0, :], m1[0:127, 2, :])
                nc.sync.dma_start(hh[0:127, 1, :], m1[1:128, 0, :])
                m1_r = m1[:].rearrange("(i s) r w -> i s r w", s=64)
                hh_r = hh[:].rearrange("(i s) r w -> i s r w", s=64)
                nc.sync.dma_start(hh_r[:, 0, 0, :], m1_r[:, 0, 0, :])
                nc.sync.dma_start(hh_r[:, 63, 1, :], m1_r[:, 63, 2, :])
                pv = strp.tile([128, 2, W], f16, tag="pv_s")
                nc.vector.tensor_max(pv[:], m1[:, 0:2, :], m1[:, 1:3, :])
                m2 = strp.tile([128, 3, W], f16, tag="m2_pre")
                nc.vector.tensor_max(m2[:, 0, :], pv[:, 0, :], hh[:, 0, :])
                nc.vector.tensor_max(m2[:, 1, :], pv[:, 0, :], pv[:, 1, :])
                nc.vector.tensor_max(m2[:, 2, :], pv[:, 1, :], hh[:, 1, :])
                return m2

            # ---- MLP over tiles ----
            xf_flat = xf[:].rearrange("p i r w -> p (i r w)")
            dxs_t = [mskp.tile([128, RPU, W], f16, tag=f"dx{i}", name=f"dx{i}") for i in range(BL)]
            dx_flats = [t[:].rearrange("p r w -> p (r w)") for t in dxs_t]

            TGROUPS = [(0, 2), (2, 2), (4, 2), (6, 2), (8, 1)]
            groups = [(img, ts0, tgn) for img in range(BL) for ts0, tgn in TGROUPS]
            GPI = len(TGROUPS)
            h1_of = {}
            h2_of = {}

            def evac(eng, dst, src, bias):
                if eng == "a":
                    nc.scalar.activation(dst, src, AF.Relu, bias=bias)
                else:
                    nc.vector.tensor_scalar(dst, src, bias, 0.0, OP.add, OP.max)

            def emit_l1_pair(gi, p):
                img, ts0, tgn = groups[gi]
                ue, uo = 2 * p, 2 * p + 1
                z1e = pz12.tile([128, 2 * TS], f32, tag="z", name=f"z1_{gi}_{ue}")
                z1o = pz12.tile([128, 2 * TS], f32, tag="z", name=f"z1_{gi}_{uo}")
                for ti in range(tgn):
                    off = (ts0 + ti) * TS
                    nc.tensor.matmul(
                        z1e[:, ti * TS:(ti + 1) * TS], w1dup[0:48, :],
                        P[0:48, img, p, off:off + TS],
                        start=True, stop=True, tile_position=(0, 0),
                    )
                    nc.tensor.matmul(
                        z1o[:, ti * TS:(ti + 1) * TS], w1dup[64:112, :],
                        P[64:112, img, p, off:off + TS],
                        start=True, stop=True, tile_position=(64, 0),
                    )
                for u, z1g in ((ue, z1e), (uo, z1o)):
                    h1g = h1p.tile([128, 2 * TS], f16, tag="h1", name=f"h1_{gi}_{u}")
                    evac(L1_ENG[u], h1g[:, 0:tgn * TS], z1g[:, 0:tgn * TS], b1c)
                    h1_of[(gi, u)] = h1g

            def emit_l2_unit(gi, u):
                img, ts0, tgn = groups[gi]
                h1g = h1_of.pop((gi, u))
                z2g = pz12.tile([128, 2 * TS], f32, tag="z", name=f"z2_{gi}_{u}")
                for ti in range(tgn):
                    nc.tensor.matmul(
                        z2g[:, ti * TS:(ti + 1) * TS], w2t[:, :],
                        h1g[:, ti * TS:(ti + 1) * TS], start=True, stop=True,
                    )
                h2g = h2p.tile([128, 2 * TS], f16, tag="h2", name=f"h2_{gi}_{u}")
                evac(L2_ENG[u], h2g[:, 0:tgn * TS], z2g[:, 0:tgn * TS], b2c)
                h2_of[(gi, u)] = h2g

            def emit_l3(gi):
                img, ts0, tgn = groups[gi]
                for ti in range(tgn):
                    t = ts0 + ti
                    off = t * TS
                    z3 = pz3.tile([128, TS], f32, tag="z3", name=f"z3_{gi}_{ti}")
                    for j in range(4):
                        nc.tensor.matmul(
                            z3[32 * j:32 * j + 32, :], w3t[:, 0:32],
                            h2_of[(gi, 2 * j)][:, ti * TS:(ti + 1) * TS],
                            start=True, stop=False, tile_position=(0, 32 * j),
                        )
                        nc.tensor.matmul(
                            z3[32 * j:32 * j + 32, :], w3t[:, 32:64],
                            h2_of[(gi, 2 * j + 1)][:, ti * TS:(ti + 1) * TS],
                            start=False, stop=True, tile_position=(0, 32 * j),
                        )
                    dsl = dx_flats[img][:, off:off + TS]
                    # dx = (z3 + b3) * umask, then += x  (x_new in place)
                    nc.vector.scalar_tensor_tensor(
                        dsl, z3[:, :], b3c, umasks[img][:, off:off + TS],
                        OP.add, OP.mult,
                    )
                    nc.gpsimd.tensor_add(
                        dsl, dsl, xf_flat[:, img * FHI + W + off:img * FHI + W + off + TS]
                    )
                for u in range(U):
                    h2_of.pop((gi, u))

            # ---- per-image epilogue (post-life maxpool on GpSimd, mul+store) ----
            lifec_d = dramp.tile([128, 3 * W], f16)
            al_post = strp.tile([128, 3, W], f16, tag="al_s")
            pm_post = strp.tile([128, 3, 191], f16, tag="pm_s")
            m1_post = strp.tile([128, 3, W], f16, tag="m1_s")
            hh_post = strp.tile([128, 2, W], f16, tag="hh_s")
            pv_post = strp.tile([128, 2, W], f16, tag="pv_s")
            m2_post = strp.tile([128, 3, W], f16, tag="m2_post")
            lifec = strp.tile([128, 3 * W], f16, tag="lifec")
            m2pre = None  # set by emit_m2pre

            def emit_epilogue(img):
                dxi = dxs_t[img]
                dx_r = dxi[:].rearrange("(u c) r w -> u c r w", c=16)
                src = dx_r[:, 3, :, :].rearrange("u (s r) w -> u s (r w)", r=3)
                sl = slice(img * 64, (img + 1) * 64)
                nc.sync.dma_start(al_post[img * 64:img * 64 + 32, :, :], src[0:4])
                nc.scalar.dma_start(al_post[img * 64 + 32:(img + 1) * 64, :, :], src[4:8])
                nc.vector.tensor_max(pm_post[sl], al_post[sl, :, 0:191], al_post[sl, :, 1:192])
                nc.vector.tensor_max(m1_post[sl, :, 1:191], pm_post[sl, :, 0:190], pm_post[sl, :, 1:191])
                nc.vector.tensor_copy(m1_post[sl, :, 0:1], pm_post[sl, :, 0:1])
                nc.vector.tensor_copy(m1_post[sl, :, 191:192], pm_post[sl, :, 190:191])
                nc.sync.dma_start(hh_post[img * 64 + 1:(img + 1) * 64, 0, :], m1_post[img * 64:(img + 1) * 64 - 1, 2, :])
                nc.sync.dma_start(hh_post[img * 64:(img + 1) * 64 - 1, 1, :], m1_post[img * 64 + 1:(img + 1) * 64, 0, :])
                nc.sync.dma_start(hh_post[img * 64:img * 64 + 1, 0, :], m1_post[img * 64:img * 64 + 1, 0, :])
                nc.sync.dma_start(hh_post[(img + 1) * 64 - 1:(img + 1) * 64, 1, :], m1_post[(img + 1) * 64 - 1:(img + 1) * 64, 2, :])
                nc.vector.tensor_max(pv_post[sl], m1_post[sl, 0:2, :], m1_post[sl, 1:3, :])
                nc.vector.tensor_max(m2_post[sl, 0, :], pv_post[sl, 0, :], hh_post[sl, 0, :])
                nc.vector.tensor_max(m2_post[sl, 1, :], pv_post[sl, 0, :], pv_post[sl, 1, :])
                nc.vector.tensor_max(m2_post[sl, 2, :], pv_post[sl, 1, :], hh_post[sl, 1, :])

                nc.vector.tensor_tensor(
                    lifec[sl], m2pre[:].rearrange("p r w -> p (r w)")[sl],
                    m2_post[:].rearrange("p r w -> p (r w)")[sl], OP.min,
                )
                nc.vector.tensor_scalar(lifec[sl], lifec[sl], 0.1, None, OP.is_gt)

                # broadcast life over channels (bounce via DRAM)
                nc.sync.dma_start(lifec_d[sl], lifec[sl])
                life = mskp.tile([128, FPI], f16, tag="life", name=f"life{img}")
                for u in range(U):
                    bsrc = lifec_d[img * 64 + 8 * u: img * 64 + 8 * u + 8, :]
                    bsrc = bsrc.rearrange("s w -> (s w)").partition_broadcast(16)
                    eng = nc.sync if u % 2 == 0 else nc.scalar
                    eng.dma_start(life[u * 16:(u + 1) * 16], bsrc)

                # final mask multiply in place (bf16) + store in 2 half-image DMAs
                nc.vector.tensor_mul(dx_flats[img], dx_flats[img], life[:])
                odst = out_d.ap()[img].rearrange("c (j r) w -> j c (r w)", r=RPU)
                nc.sync.dma_start(odst[0:4], dx_flats[img][0:64, :])
                nc.scalar.dma_start(odst[4:8], dx_flats[img][64:128, :])

            # ---- pipelined driver ----
            emit_chunk(0, 0, 6)
            emit_chunk(0, 6, 6)
            emit_chunk(0, 12, 6)
            CHUNK_SCHED = {
                0: [(0, 18, 6)], 1: [(1, 0, 6)], 2: [(1, 6, 6)],
                3: [(1, 12, 6)], 4: [(1, 18, 6)],
            }
            for gi in range(len(groups) + 1):
                for cimg, cr0, crc in CHUNK_SCHED.get(gi, []):
                    emit_chunk(cimg, cr0, crc)
                if gi == 0:
                    emit_umask(0)
                if gi == 1:
                    emit_umask(1)
                    m2pre = emit_m2pre()
                for p in range(4):
                    if gi < len(groups):
                        emit_l1_pair(gi, p)
                    if gi >= 1:
                        emit_l2_unit(gi - 1, 2 * p)
                        emit_l2_unit(gi - 1, 2 * p + 1)
                if gi >= 1:
                    emit_l3(gi - 1)
                    if (gi % GPI) == 0:
                        emit_epilogue(gi // GPI - 1)

    nc.compile()
    return nc


def host_prep(inputs):
    """Full inputs -> list of 8 per-core input dicts."""
    x = np.ascontiguousarray(inputs["x"], dtype=np.float32)
    fn = np.ascontiguousarray(inputs["fire_noise"], dtype=np.float32)
    w1 = np.asarray(inputs["w1"], np.float32)
    b1 = np.asarray(inputs["b1"], np.float32)
    w2 = np.asarray(inputs["w2"], np.float32)
    b2 = np.asarray(inputs["b2"], np.float32)
    w3 = np.asarray(inputs["w3"], np.float32)
    b3 = np.asarray(inputs["b3"], np.float32)

    # lhsT rows 0:16 = w1a.T, 16:32 = w1b.T/8, 32:48 = w1c.T/8, dup at +64
    w1dup = np.zeros((128, 128), ml_dtypes.bfloat16)
    stack = np.concatenate(
        [w1[:, 0:16].T, w1[:, 16:32].T / 8.0, w1[:, 32:48].T / 8.0], axis=0
    ).astype(ml_dtypes.bfloat16)  # [48, 128]
    w1dup[0:48] = stack
    w1dup[64:112] = stack
    w2t = w2.T.astype(ml_dtypes.bfloat16)
    w3t = np.zeros((128, 64), ml_dtypes.bfloat16)
    w3t[:, 0:16] = w3.T.astype(ml_dtypes.bfloat16)
    w3t[:, 48:64] = w3.T.astype(ml_dtypes.bfloat16)
    b3col = np.tile(b3, U).reshape(128).astype(np.float32)

    wm = np.concatenate([w1dup, w2t, w3t], axis=1)  # [128, 320]
    bm = np.stack([
        b1.astype(np.float32), b2.astype(np.float32), b3col
    ], axis=1)  # [128, 3]
    shared = {"wm": wm, "bm": bm}
    xh = x.astype(ml_dtypes.bfloat16)
    um = (fn[:, 0] <= 0.5).astype(ml_dtypes.bfloat16)
    in_maps = []
    for core in range(8):
        m = dict(shared)
        m["x"] = xh[2 * core:2 * core + 2]
        m["fn"] = um[2 * core:2 * core + 2]
        in_maps.append(m)
    return in_maps


_NC_CACHE = None


def kernel(**inputs):
    global _NC_CACHE
    from concourse.bass_utils import run_bass_kernel_spmd
    if _NC_CACHE is None:
        _NC_CACHE = build_nc()
    in_maps = host_prep(inputs)
    res = run_bass_kernel_spmd(_NC_CACHE, in_maps, core_ids=list(range(8)))
    return np.concatenate(
        [np.asarray(res.results[i]["out"], dtype=np.float32) for i in range(8)], axis=0
    )
